# revision 31
# baseline (speedup 1.0000x reference)
"""Trainium2 Bass kernel for GQA causal sliding-window self-attention.

Sharding: 8 cores = 2 (batch) x 4 (KV-head groups). Each core handles one
batch element and one KV head with its 3 GQA query heads. The output
projection is computed per-group against the matching Wproj column slice;
the 4 partial outputs per batch are summed on the host.

Layout: feature-major ("transposed") on chip; bf16 operands everywhere
(halves DMA traffic and enables the DVE 2x packed mode on the rope chain)
with fp32 PSUM accumulation. Sliding-window trimming is exact at 128-col
granularity, with triangular mask bands added via identity-matmuls into
the scores PSUM.

Schedule: software-pipelined at two levels. Across t-chunks, the
projections for chunk tci+1 interleave with attention for chunk tci and
the output projection. Within attention, the softmax-denominator and P@V
matmuls of each head are deferred through a global carry queue and emitted
between the next head's (or next stream's) matmuls, so the PE never
head-of-line blocks on the Act-engine exp. The rmsnorm row chains of the
k/q1/q2 streams are batched into single [3,TC] activations.
"""

import os
import sys
import numpy as np

sys.path.insert(0, "/opt/trn_rl_repo")

from contextlib import ExitStack

import ml_dtypes

from concourse import mybir, bacc, tile, bass_isa
from concourse.bass_utils import run_bass_kernel_spmd

f32 = mybir.dt.float32
f32r = mybir.dt.float32r
bf16 = mybir.dt.bfloat16
AF = mybir.ActivationFunctionType

B, T, C = 2, 2048, 1536
H, KV, D = 12, 4, 128
REP = H // KV          # 3 query heads per kv head
QD = REP * D           # 384
VE_GATE_CH = 12
N_CORES = 8
TC = 512               # t-chunk width (matmul moving free dim)
NTC = T // TC          # 4
NCC = C // 128         # 12 contraction chunks
NST = T // 128         # 16 s-tiles

_EPS = float(np.finfo(np.float32).eps)
# all scale constants folded into the q-side rsqrt:
#   rq = (1.2*1.2/sqrt(D)) * rsqrt(mean(q^2)+eps),  rk = rsqrt(mean(k^2)+eps)
_LNCQ = float(np.log(1.2 * 1.2 / np.sqrt(D)))
_MASKVAL = -1.0e9

_CACHE = {}


def _setup_act_tables():
    """Reorder activation-table sets so ln+exp share one set (avoids ~33
    table reloads).  Patches both the bacc-side set picker and the walrus
    --act-root-json (they must agree on set indices)."""
    try:
        import json
        import tempfile
        import concourse.hw_specs as hw_specs
        import concourse.bacc as bacc_mod
        from neuronxcc.driver.Job import Job
        from neuronxcc.driver.jobs.support.FindActInfo import findActInfoFile

        src = findActInfoFile(Job.getPackageDir(), "gen3")
        if not src or not os.path.exists(src):
            return
        src_dir = os.path.dirname(src)
        dst = os.path.join(tempfile.gettempdir(), "bass_act_pwp_lnexp")
        os.makedirs(dst, exist_ok=True)
        for f in os.listdir(src_dir):
            tgt = os.path.join(dst, f)
            if not os.path.exists(tgt):
                try:
                    os.symlink(os.path.join(src_dir, f), tgt)
                except OSError:
                    pass
        d = json.load(open(src))
        sets = d["act_func_sets"]
        idx = [i for i, s in enumerate(sets)
               if s["name"] == "natural_log_exp_and_others"]
        if not idx:
            return
        sets.insert(0, sets.pop(idx[0]))
        jp = os.path.join(dst, "act_info.json")
        if os.path.lexists(jp):
            os.remove(jp)
        json.dump(d, open(jp, "w"))
        os.environ["BASS_ACT_ROOT_JSON_PATH"] = jp

        orig = hw_specs.get_activation_tables

        def reordered(arch):
            t = orig(arch)
            key = "natural_log_exp_and_others"
            if key in t:
                out = {key: t[key]}
                out.update((k, v) for k, v in t.items() if k != key)
                return out
            return t

        hw_specs.get_activation_tables = reordered
        bacc_mod.get_activation_tables = reordered
    except Exception:
        pass


_setup_act_tables()


def _build(window: int):
    win_finite = 0 <= window < T

    nc = bacc.Bacc("TRN2", target_bir_lowering=False, debug=False,
                   num_devices=N_CORES)

    xT = nc.dram_tensor("xT", [C, T], bf16, kind="ExternalInput")
    wqT = nc.dram_tensor("wqT", [C, QD], bf16, kind="ExternalInput")
    wkvT = nc.dram_tensor("wkvT", [C, 2 * D], bf16, kind="ExternalInput")
    wpT = nc.dram_tensor("wpT", [QD, C], bf16, kind="ExternalInput")
    csvT = nc.dram_tensor("csvT", [128, 3, T], bf16, kind="ExternalInput")
    constsB = nc.dram_tensor("constsB", [128, 386], bf16,
                             kind="ExternalInput")
    outT = nc.dram_tensor("outT", [C, T], bf16, kind="ExternalOutput")

    xT_re = xT.ap().rearrange("(cc p) t -> p cc t", p=128)

    with tile.TileContext(nc) as tc, ExitStack() as ctx:
        # ---- persistent SBUF pools ----
        pw = ctx.enter_context(tc.tile_pool(name="pw", bufs=1))
        pbig = ctx.enter_context(tc.tile_pool(name="pbig", bufs=1))
        p1w = ctx.enter_context(tc.tile_pool(name="p1w", bufs=1))
        pxt = ctx.enter_context(tc.tile_pool(name="pxt", bufs=2))
        pcs = ctx.enter_context(tc.tile_pool(name="pcs", bufs=2))
        ptmp = ctx.enter_context(tc.tile_pool(name="ptmp", bufs=12))
        prow = ctx.enter_context(tc.tile_pool(name="prow", bufs=6))
        pbc = ctx.enter_context(tc.tile_pool(name="pbc", bufs=4))
        pP = ctx.enter_context(tc.tile_pool(name="pP", bufs=8))
        pout = ctx.enter_context(tc.tile_pool(name="pout", bufs=2))

        # ---- PSUM pools (8 banks total) ----
        psSY = ctx.enter_context(tc.tile_pool(name="psSY", bufs=5, space="PSUM"))
        psR = ctx.enter_context(tc.tile_pool(name="psR", bufs=2, space="PSUM"))
        psAO = ctx.enter_context(tc.tile_pool(name="psAO", bufs=1, space="PSUM"))

        # ---- constants (tiles here; DMAs emitted in the schedule below so
        # the SP queue leads with the critical wk/x loads) ----
        eps_col = pw.tile([128, 1], f32, tag="epsc")
        nc.vector.memset(eps_col[:], _EPS)
        lncq_col = pw.tile([128, 1], f32, tag="lncqc")
        nc.vector.memset(lncq_col[:], _LNCQ)
        # per-partition exp bias for the batched rmsnorm rows: matmul row
        # outputs must land on partition 0/32/64, so the k/q1/q2 rows live
        # at those partitions of one PSUM bank (k at 0 -> bias 0, q at
        # 32/64 -> bias lncq; the partitions between are don't-care).
        bias3_col = pw.tile([65, 1], f32, tag="b3c")
        nc.vector.memset(bias3_col[:], _LNCQ)
        nc.vector.memset(bias3_col[0:1, :], 0.0)
        constsB_sb = pw.tile([128, 386], bf16, tag="cB")
        eyeB_sb = constsB_sb[:, 0:128]
        tri_sb = constsB_sb[:, 128:384].rearrange("p (two q) -> p two q",
                                                  two=2)
        onesB_sb = constsB_sb[:, 384:385]
        wgB_sb = constsB_sb[0:VE_GATE_CH, 385:386]

        # ---- big persistent activations (phase-2 operands in bf16),
        # split per t-chunk so the cross-chunk pipeline has no false
        # whole-tensor dependencies ----
        qT_sb = [[pbig.tile([128, TC], bf16, tag=f"qT{m}_{i}",
                            name=f"qT{m}_{i}") for i in range(NTC)]
                 for m in range(REP)]
        kT_sb = [pbig.tile([128, TC], bf16, tag=f"kT{i}", name=f"kT{i}")
                 for i in range(NTC)]
        V_sb = [pbig.tile([128, TC // 128, D], bf16, tag=f"V{i}",
                          name=f"V{i}") for i in range(NTC)]
        yT_sb = [[pbig.tile([128, TC], bf16, tag=f"yT{m}_{i}",
                            name=f"yT{m}_{i}") for i in range(NTC)]
                 for m in range(REP)]

        # ---- weights ----
        wkv_sb = p1w.tile([128, NCC, 2 * D], bf16, tag="wkv")
        wkvT_re = wkvT.ap().rearrange("(cc p) m -> p cc m", p=128)
        wq_sb = p1w.tile([128, NCC, QD], bf16, tag="wq")
        wqT_re = wqT.ap().rearrange("(cc p) m -> p cc m", p=128)
        wp_sb = p1w.tile([128, REP, C], bf16, tag="wp")

        # ---- global PE carry: deferred den/PV + finalize closures from
        # attention heads, drained one per independent PE matmul ----
        pe_carry = []

        def tick(n=1):
            k = len(pe_carry) if n is None else min(n, len(pe_carry))
            for _ in range(k):
                pe_carry.pop(0)()

        xt_tiles = {}

        def load_xt(tci, groups=(0, 4, 8), width=4):
            t0 = tci * TC
            if tci not in xt_tiles:
                xt_tiles[tci] = pxt.tile([128, NCC, TC], bf16, tag="xt",
                                         name="xth")
            xth = xt_tiles[tci]
            for g0 in groups:
                nc.sync.dma_start(xth[:, g0:g0 + width, :],
                                  xT_re[:, g0:g0 + width, t0:t0 + TC])

        csv_tiles = {}

        def load_csv(tci, parts=((0, 3),)):
            t0 = tci * TC
            if tci not in csv_tiles:
                csv = pcs.tile([128, 3, TC], bf16, tag="csv", name="csv")
                csv_tiles[tci] = (csv[:, 0, :], csv[:, 1, :], csv[:, 2, :])
                csv_tiles[tci] += (csv,)
            csv = csv_tiles[tci][3]
            for a, b in parts:
                nc.sync.dma_start(csv[:, a:b, :],
                                  csvT.ap()[:, a:b, t0:t0 + TC])

        # deferred per-stream epilogues (rmsnorm+rope chains, v transposes)
        pending = []

        def flush(n=None):
            k = len(pending) if n is None else min(n, len(pending))
            for _ in range(k):
                pending.pop(0)()

        gate_bc = {}

        def gate(tci):
            """ve gate: 3*sigmoid(x[:, :12] @ wg) -- the *3 folded into veT."""
            xt = xt_tiles[tci]
            zg = psR.tile([1, TC], f32, tag="row", name="zg")
            nc.tensor.matmul(zg[0:1, :], wgB_sb,
                             xt[0:VE_GATE_CH, 0, :], start=True, stop=True)
            tick(1)
            ez = prow.tile([1, TC], f32, tag="g", name="ez")
            nc.scalar.activation(ez[:], zg[:], AF.Exp, scale=-1.0)
            ez1 = prow.tile([1, TC], f32, tag="g", name="ez1")
            nc.vector.tensor_scalar_add(ez1[:], ez[:], 1.0)
            grow = prow.tile([1, TC], bf16, tag="g", name="grow")
            with nc.allow_low_precision(reason="sigmoid gate in bf16 is ok"):
                nc.vector.reciprocal(grow[:], ez1[:])
            gbc = pbc.tile([128, TC], bf16, tag="bc", name="gbc")
            nc.gpsimd.partition_broadcast(gbc[:], grow[:])
            gate_bc[tci] = gbc

        ss3_tiles = {}
        part2_args = {}

        def stream(tci, kind, m=0, slot=None):
            """One projection stream: matmuls into PSUM + deferred epilogue.

            slot=None -> self-contained rmsnorm row chain (used for chunk-0 k
            and every q0).  slot=i -> the ones-matmul writes row i of the
            shared per-chunk [3,TC] PSUM tile; ln/exp run batched later in
            qk_batch()."""
            xt = xt_tiles[tci]
            acc = psSY.tile([128, TC], f32, tag="sy", name="acc")
            for cc in range(NCC):
                if kind == "q":
                    lhsT = wq_sb[:, cc, m * D:(m + 1) * D]
                elif kind == "k":
                    lhsT = wkv_sb[:, cc, 0:D]
                else:
                    lhsT = wkv_sb[:, cc, D:2 * D]
                nc.tensor.matmul(
                    acc[:], lhsT, xt[:, cc, :],
                    start=(cc == 0), stop=(cc == NCC - 1))
                if cc in (2, 5, 8):
                    tick(1)

            if kind == "v":
                # v += gate * ve; then transpose into natural [s, D] (bf16)
                gbc = gate_bc.pop(tci)
                ve_t = csv_tiles[tci][2]
                vtmp = ptmp.tile([128, TC], bf16, tag="t", name="vtmp")
                nc.vector.tensor_mul(vtmp[:], gbc[:], ve_t[:])
                vfull = ptmp.tile([128, TC], bf16, tag="t", name="vfull")
                nc.vector.tensor_add(vfull[:], vtmp[:], acc[:])

                def vtrans(tci=tci, vfull=vfull):
                    for j in range(TC // 128):
                        vtr = psSY.tile([128, 128], bf16, tag="sy", name="vtr")
                        nc.tensor.transpose(
                            vtr[:], vfull[:, j * 128:(j + 1) * 128], eyeB_sb)
                        if j % 2 == 0:
                            nc.scalar.copy(V_sb[tci][:, j, :], vtr[:])
                        else:
                            nc.vector.tensor_copy(V_sb[tci][:, j, :], vtr[:])
                pending.append(vtrans)
                return

            # q/k epilogue: evacuate + square immediately (square reads the
            # PSUM directly on Act while the DVE copy drains it); the rope
            # half-swap DMA runs early on the Act DGE queue so it is off the
            # rsqrt-chain critical path.
            cs, sn = csv_tiles[tci][0], csv_tiles[tci][1]
            qraw = ptmp.tile([128, TC], bf16, tag="t", name="qraw")
            nc.vector.tensor_copy(qraw[:], acc[:])
            sqr = ptmp.tile([128, TC], bf16, tag="t", name="sqr")
            nc.scalar.activation(sqr[:], acc[:], AF.Square)
            qsw = ptmp.tile([128, TC], bf16, tag="t", name="qsw")
            nc.sync.dma_start(qsw[0:64, :], qraw[64:128, :])
            nc.sync.dma_start(qsw[64:128, :], qraw[0:64, :])
            ta = ptmp.tile([128, TC], bf16, tag="t", name="ta")
            nc.vector.tensor_mul(ta[:], qraw[:], cs[:])

            if slot is not None:
                part2_args[(tci, slot)] = (kind, m, qsw, ta, sn)

                def ssmm(tci=tci, slot=slot, sqr=sqr):
                    if tci not in ss3_tiles:
                        ss3_tiles[tci] = psR.tile([65, TC], f32, tag="row",
                                                  name="ss3")
                        if os.environ.get("BASS_SS3_INIT") == "1":
                            # CoreSim-only: the batched ln/exp reads the
                            # don't-care partitions between the rows, which
                            # trips the simulator's uninitialized-read check.
                            nc.vector.memset(ss3_tiles[tci][1:32, :], 1.0)
                            nc.vector.memset(ss3_tiles[tci][33:64, :], 1.0)
                    p = 32 * slot
                    nc.tensor.matmul(ss3_tiles[tci][p:p + 1, :],
                                     onesB_sb, sqr[:], start=True, stop=True)
                pending.append(ssmm)
                return

            def final(kind=kind, m=m, tci=tci, sqr=sqr,
                      qsw=qsw, ta=ta, sn=sn):
                ss = psR.tile([1, TC], f32, tag="row", name="ss")
                nc.tensor.matmul(ss[:], onesB_sb, sqr[:],
                                 start=True, stop=True)
                lnr = prow.tile([1, TC], f32, tag="r", name="lnr")
                nc.scalar.activation(lnr[:], ss[:], AF.Ln,
                                     scale=1.0 / D, bias=eps_col[0:1, :])
                rr = prow.tile([1, TC], f32, tag="r", name="rr")
                if kind == "q":
                    nc.scalar.activation(rr[:], lnr[:], AF.Exp, scale=-0.5,
                                         bias=lncq_col[0:1, :])
                else:
                    nc.scalar.activation(rr[:], lnr[:], AF.Exp, scale=-0.5,
                                         bias=0.0)
                rbc = pbc.tile([128, TC], f32, tag="bc", name="rbc")
                nc.gpsimd.partition_broadcast(rbc[:], rr[:])
                tb = ptmp.tile([128, TC], bf16, tag="t", name="tb")
                nc.vector.tensor_mul(tb[:], qsw[:], sn[:])
                u = ptmp.tile([128, TC], bf16, tag="t", name="u")
                nc.vector.tensor_add(u[:], ta[:], tb[:])
                dst = qT_sb[m][tci] if kind == "q" else kT_sb[tci]
                nc.vector.tensor_mul(dst[:], u[:], rbc[:])
            pending.append(final)

        def qk_batch(tci, q_only):
            """Batched ln/exp over the chunk's rmsnorm rows (at partitions
            0/32/64 of one PSUM bank -- matmul base-partition rule; an AP
            with nonzero partition offset may span at most 32 partitions,
            so slot 2 gets its own pair of row activations)."""
            ss3 = ss3_tiles.pop(tci)
            rr3 = prow.tile([65, TC], f32, tag="r3", name="rr3")
            # slots 0,1 (partitions 0..32) in one act pair
            ln3 = prow.tile([65, TC], f32, tag="r3", name="ln3")
            nc.scalar.activation(ln3[0:33, :], ss3[0:33, :], AF.Ln,
                                 scale=1.0 / D, bias=eps_col[0:33, :])
            bias01 = lncq_col[0:33, :] if q_only else bias3_col[0:33, :]
            nc.scalar.activation(rr3[0:33, :], ln3[0:33, :], AF.Exp,
                                 scale=-0.5, bias=bias01)
            if (tci, 2) in part2_args:
                nc.scalar.activation(ln3[64:65, :], ss3[64:65, :], AF.Ln,
                                     scale=1.0 / D, bias=eps_col[0:1, :])
                nc.scalar.activation(rr3[64:65, :], ln3[64:65, :], AF.Exp,
                                     scale=-0.5, bias=lncq_col[0:1, :])
            for slot in (0, 1, 2):
                if (tci, slot) not in part2_args:
                    continue
                kind, m, qsw, ta, sn = part2_args.pop((tci, slot))
                p = 32 * slot
                if p:
                    # partition_broadcast's ucode broadcasts physical
                    # partition 0 -- move offset rows down via DMA first
                    rrow = prow.tile([1, TC], f32, tag="r", name="rrow")
                    nc.sync.dma_start(rrow[:], rr3[p:p + 1, :])
                else:
                    rrow = rr3[0:1, :]
                rbc = pbc.tile([128, TC], f32, tag="bc", name="rbc")
                nc.gpsimd.partition_broadcast(rbc[:], rrow[:])
                tb = ptmp.tile([128, TC], bf16, tag="t", name="tb")
                nc.vector.tensor_mul(tb[:], qsw[:], sn[:])
                u = ptmp.tile([128, TC], bf16, tag="t", name="u")
                nc.vector.tensor_add(u[:], ta[:], tb[:])
                dst = qT_sb[m][tci] if kind == "q" else kT_sb[tci]
                nc.vector.tensor_mul(dst[:], u[:], rbc[:])

        def attn(tci, h):
            t0 = tci * TC
            if win_finite:
                st_min = max(0, (t0 - window - 127) // 128 + 1)
            else:
                st_min = 0
            st_max = (t0 + TC - 1) // 128
            st_diag = t0 // 128
            sts = [st_diag] + [s for s in range(st_min, st_max + 1)
                               if s != st_diag]
            n = len(sts)

            yU = psSY.tile([128, TC], f32, tag="sy", name="yU")
            den = psR.tile([1, TC], f32, tag="row", name="den")
            pends = []
            for idx, st in enumerate(sts):
                s0 = st * 128
                delta = t0 - s0
                causal_p = delta <= 0
                window_p = win_finite and delta > window - (TC - 1)
                v0 = -delta if causal_p else 0
                v1 = min(TC, window - delta + 128) if window_p else TC
                sc = psSY.tile([128, TC], f32, tag="sy", name="sc")
                kk = kT_sb[st // (TC // 128)]
                ko = (st % (TC // 128)) * 128
                nc.tensor.matmul(sc[:, v0:v1], kk[:, ko:ko + 128],
                                 qT_sb[h][tci][:, v0:v1],
                                 start=True,
                                 stop=(not causal_p and not window_p))
                if causal_p:    # 128-wide staircase band at [v0, v0+128)
                    nc.tensor.matmul(sc[:, v0:v0 + 128], eyeB_sb,
                                     tri_sb[:, 0, :], start=False, stop=True)
                if window_p:    # 128-wide staircase band at [v1-128, v1)
                    nc.tensor.matmul(sc[:, v1 - 128:v1], eyeB_sb,
                                     tri_sb[:, 1, :], start=False, stop=True)
                tick(1)
                if len(pends) >= 3:
                    pends.pop(0)()
                P = pP.tile([128, TC], bf16, tag="P", name="P")
                nc.scalar.activation(P[:, v0:v1], sc[:, v0:v1], AF.Exp)

                def mk(idx=idx, st=st, P=P, v0=v0, v1=v1):
                    first, last = idx == 0, idx == n - 1

                    def go():
                        nc.tensor.matmul(den[0:1, v0:v1], onesB_sb,
                                         P[:, v0:v1], start=first, stop=last)
                        nc.tensor.matmul(
                            yU[:, v0:v1],
                            V_sb[st // (TC // 128)][:, st % (TC // 128), :],
                            P[:, v0:v1], start=first, stop=last)
                    return go
                pends.append(mk())

            def finalize(tci=tci, h=h, yU=yU, den=den):
                dinv = prow.tile([1, TC], f32, tag="r", name="dinv")
                nc.vector.reciprocal(dinv[:], den[:])
                dbc = pbc.tile([128, TC], f32, tag="bc", name="dbc")
                nc.gpsimd.partition_broadcast(dbc[:], dinv[:])
                nc.vector.tensor_mul(yT_sb[h][tci][:], dbc[:], yU[:])
            assert not pe_carry, "old carry must drain before re-filling"
            pe_carry.extend(pends)
            pe_carry.append(finalize)

        outT_re = outT.ap().rearrange("(g two p) t -> p (g two) t",
                                      p=128, two=2)

        oq_state = {}

        def outproj(tci, ccs=range(NCC)):
            t0 = tci * TC
            ot_quad = oq_state.pop(tci, None)
            for cc in ccs:
                pool = psAO if cc % 3 == 0 else psSY
                o = pool.tile([128, TC], f32, tag="ao" if cc % 3 == 0
                              else "sy", name="o")
                for m in range(REP):
                    nc.tensor.matmul(
                        o[:], wp_sb[:, m, cc * 128:(cc + 1) * 128],
                        yT_sb[m][tci][:],
                        start=(m == 0), stop=(m == REP - 1))
                tick(1)
                # evacuations alternate engines so two banks drain in
                # parallel; out DMAs go out as 4-cc quads (HWDGE descriptor
                # generation is the scarce resource, not bandwidth)
                q = cc % 4
                if q == 0:
                    ot_quad = pout.tile([128, 4, TC], bf16, tag="ot",
                                        name="ot")
                if cc % 2 == 0:
                    nc.scalar.copy(ot_quad[:, q, :], o[:])
                else:
                    nc.vector.tensor_copy(ot_quad[:, q, :], o[:])
                if q == 3:
                    nc.sync.dma_start(
                        outT_re[:, cc - 3:cc + 1, t0:t0 + TC], ot_quad[:])
            oq_state[tci] = ot_quad

        def outproj_last(tci):
            """Last chunk: pre-issue the m0/m1 partials of the first ccs so
            the PE carry (den/PV + finalize of the last head) drains without
            head-of-line blocking, then stream out per-cc on both queues."""
            t0 = tci * TC
            o_tiles = {}
            pre = 3
            for cc in range(pre):
                pool = psAO if cc == 0 else psSY
                o = pool.tile([128, TC], f32, tag="ao" if cc == 0
                              else "sy", name="o")
                o_tiles[cc] = o
                for m in range(2):
                    nc.tensor.matmul(
                        o[:], wp_sb[:, m, cc * 128:(cc + 1) * 128],
                        yT_sb[m][tci][:],
                        start=(m == 0), stop=False)
                tick(1)
            tick(None)

            # the last chunk ships small pair-groups as soon as each is
            # evacuated, so the exclusive DMA-transfer pool drains in
            # parallel with the remaining trios and the final DMA is tiny
            groups = {1: 2, 3: 2, 5: 2, 7: 2, 9: 2, 10: 1, 11: 1}
            otl = {}

            def evac(cc, o):
                if not otl:
                    otl["t"] = pout.tile([128, 2, TC], bf16, tag="otl",
                                         name="ot", bufs=6)
                    otl["lo"] = cc
                q = cc - otl["lo"]
                if cc % 2 == 0:
                    nc.scalar.copy(otl["t"][:, q, :], o[:])
                else:
                    nc.vector.tensor_copy(otl["t"][:, q, :], o[:])
                if cc in groups:
                    n = groups[cc]
                    nc.sync.dma_start(
                        outT_re[:, cc - n + 1:cc + 1, t0:t0 + TC],
                        otl["t"][:, q - n + 1:q + 1, :])
                    otl.clear()

            for cc in range(pre):
                o = o_tiles.pop(cc)
                nc.tensor.matmul(
                    o[:], wp_sb[:, 2, cc * 128:(cc + 1) * 128],
                    yT_sb[2][tci][:], start=False, stop=True)
                evac(cc, o)
            for cc in range(pre, NCC):
                pool = psAO if cc in (6, 9) else psSY
                o = pool.tile([128, TC], f32, tag="ao" if cc in (6, 9)
                              else "sy", name="o")
                for m in range(REP):
                    nc.tensor.matmul(
                        o[:], wp_sb[:, m, cc * 128:(cc + 1) * 128],
                        yT_sb[m][tci][:],
                        start=(m == 0), stop=(m == REP - 1))
                evac(cc, o)

        # ================= emission schedule =================
        # Prologue loads are split across the two HWDGE queues (SP + Act)
        # so the chunk-0 projection streams are fed ~2x faster; afterwards
        # inputs ride SP and outputs + rope swaps ride the Act queue.
        nc.sync.dma_start(constsB_sb[:], constsB.ap()[:])
        nc.sync.dma_start(wkv_sb[:, 0:2, :], wkvT_re[:, 0:2, :])
        load_xt(0, groups=(0,), width=2)
        load_csv(0, parts=((0, 2),))
        nc.scalar.dma_start(wkv_sb[:, 2:7, :], wkvT_re[:, 2:7, :])
        load_xt(0, groups=(2,), width=3)
        nc.scalar.dma_start(wkv_sb[:, 7:12, :], wkvT_re[:, 7:12, :])
        load_xt(0, groups=(5,), width=4)
        nc.scalar.dma_start(wq_sb[:, 0:4, :], wqT_re[:, 0:4, :])
        load_xt(0, groups=(9,), width=3)
        load_csv(0, parts=((2, 3),))
        nc.scalar.dma_start(wq_sb[:, 4:8, :], wqT_re[:, 4:8, :])
        nc.scalar.dma_start(wq_sb[:, 8:12, :], wqT_re[:, 8:12, :])
        load_xt(1, groups=(0,), width=NCC)
        nc.sync.dma_start(wp_sb[:], wpT.ap().rearrange("(qc p) c -> p qc c",
                                                       p=128))

        stream(0, "k")          # pending: [kf0]
        gate(0)
        stream(0, "v")          # pending: [kf0, vtrans(0)]
        stream(0, "q", 0)
        flush(2)                # kf0, vtrans(0)
        stream(0, "q", 1, slot=0)
        flush(1)                # q0 final
        stream(0, "q", 2, slot=1)
        flush(2)                # ssmm q1, ssmm q2
        qk_batch(0, True)

        for tci in range(NTC):
            nxt = tci + 1
            attn(tci, 0)
            if nxt < NTC:
                if nxt + 1 < NTC:
                    load_xt(nxt + 1, groups=(0,), width=NCC)
                load_csv(nxt)
                stream(nxt, "k", slot=0)    # pending: [ssmm k]
            attn(tci, 1)
            if nxt < NTC:
                gate(nxt)
                stream(nxt, "v")            # pending: [ssmm k, vtrans]
                stream(nxt, "q", 0)         # pending: [.., .., q0 final]
            attn(tci, 2)
            if nxt < NTC:
                flush(2)                    # ssmm k, vtrans(nxt)
                tick(None)                  # drain head-2 carry fully
                stream(nxt, "q", 1, slot=1)
                flush(1)                    # q0(nxt) final
                stream(nxt, "q", 2, slot=2)
                flush(2)                    # ssmm q1, ssmm q2
                qk_batch(nxt, False)
                outproj(tci)                # 12 trios cover the qk chain
            else:
                outproj_last(tci)
        flush()
        assert not pe_carry and not pending

    nc.compile()
    return nc


def _prep_inputs(nc, window, x, ve, cos, sin, Wq, Wk, Wv, Wproj, Wg):
    """Build the 8 per-core input maps (host-side sharding + transposes)."""
    cosT = np.ascontiguousarray(cos.reshape(T, D // 2).T)
    sinT = np.ascontiguousarray(sin.reshape(T, D // 2).T)
    cos2 = np.concatenate([cosT, cosT], axis=0)
    sin2m = np.concatenate([sinT, -sinT], axis=0)

    # 128-wide triangular staircase mask bands (additive, pre-exp):
    #   tri[:, 0, :] causal band: valid when p <= j
    #   tri[:, 1, :] window band: valid when p >= j
    p = np.arange(128)[:, None]
    j = np.arange(128)[None, :]
    tri = np.empty((128, 2, 128), dtype=np.float32)
    tri[:, 0, :] = np.where(p <= j, 0.0, _MASKVAL)
    tri[:, 1, :] = np.where(p >= j, 0.0, _MASKVAL)

    constsB = np.zeros((128, 386), dtype=np.float32)
    constsB[:, 0:128] = np.eye(128)
    constsB[:, 128:384] = tri.reshape(128, 256)
    constsB[:, 384] = 1.0

    xTb = [np.ascontiguousarray(x[b].T).astype(ml_dtypes.bfloat16)
           for b in range(B)]

    in_maps = []
    for core in range(N_CORES):
        b, g = divmod(core, KV)
        sl_q = slice(g * QD, (g + 1) * QD)
        sl_d = slice(g * D, (g + 1) * D)
        cB = constsB.copy()
        cB[:VE_GATE_CH, 385] = Wg[g]
        in_maps.append({
            "xT": xTb[b],
            "wqT": np.ascontiguousarray(Wq[sl_q].T).astype(ml_dtypes.bfloat16),
            "wkvT": np.ascontiguousarray(
                np.concatenate([Wk[sl_d].T, Wv[sl_d].T], axis=1)).astype(
                ml_dtypes.bfloat16),
            "wpT": np.ascontiguousarray(Wproj[:, sl_q].T).astype(
                ml_dtypes.bfloat16),
            "csvT": np.stack(
                [cos2, sin2m, 3.0 * ve[b, :, sl_d].T], axis=1).astype(
                ml_dtypes.bfloat16),
            "constsB": cB.astype(ml_dtypes.bfloat16),
        })
    return in_maps


def kernel(x, ve, cos, sin, Wq, Wk, Wv, Wproj, Wg, window, _trace=False):
    window = int(window)
    if window not in _CACHE:
        _CACHE[window] = _build(window)
    nc = _CACHE[window]

    in_maps = _prep_inputs(nc, window,
                           np.asarray(x, np.float32), np.asarray(ve, np.float32),
                           np.asarray(cos, np.float32), np.asarray(sin, np.float32),
                           np.asarray(Wq, np.float32), np.asarray(Wk, np.float32),
                           np.asarray(Wv, np.float32), np.asarray(Wproj, np.float32),
                           np.asarray(Wg, np.float32))

    res = run_bass_kernel_spmd(nc, in_maps, core_ids=list(range(N_CORES)),
                               trace=_trace)

    out = np.empty((B, T, C), dtype=np.float32)
    for b in range(B):
        acc = res.results[b * KV]["outT"].astype(np.float32)
        for g in range(1, KV):
            acc += res.results[b * KV + g]["outT"].astype(np.float32)
        out[b] = acc.T
    if _trace:
        kernel._last_trace = res
    return out
